# revision 3
# baseline (speedup 1.0000x reference)
"""Bass/Tile kernel for nn_MAlphaAttention (banded sparse graph attention).

Sharding: 8 cores = 4 batches x 2 head-groups (6 heads each); host sums
the two per-batch partials and adds b_out.

Key structure: the graph mask M (2D 32x32 grid, order-5 diffusion) is
banded: M[n,m] != 0 only for |n-m| <= 160. G = I + 0.1*M shares the
band. All N x N stages are computed banded:
  P1  qkv^T projection in fp8-e4m3 DoubleRow with hi/lo error
      compensation: x = (x8 + r8)/sx, W = (w8 + rw8)/sw; psum
      accumulates x8.w8 (3 DR matmuls pairing kc chunks) plus the
      mixed-residual terms x8.rw8 + r8.w8 (6 DR matmuls, one per kc,
      using the DR plane-pair to evaluate both corrections at once).
      The dropped r8.rw8 term is ~0.1% relative.  relu/copy drains
      scale by 1/(sx*sw).
  P2  graph mix fused with n->d transpose, banded over 256-col quarters
      with edge-trimmed windows: qT[d,m] = sum_n q[n,d] G[n,m].
  P3  per head h, m-chunk mc: S^T[m, n-window(mc)] (K=64 matmul over
      the <=448-wide band window); A^T = S^T * maskT on DVE (banded);
      O[n-chunk, d|z] = sum_{mc in band} A^T(mc) @ [v|1], accumulated
      with partial-partition matmuls at the band edges; z = 1/sum via
      DVE reciprocal; z-scale on ACT/DVE drain; O transposed to
      d-major via PE identity matmul.
"""

import numpy as np
from contextlib import ExitStack

import concourse.bass as bass
from concourse import bacc
import concourse.tile as tile
import concourse.mybir as mybir
from concourse.bass_utils import run_bass_kernel_spmd

F32 = mybir.dt.float32
BF16 = mybir.dt.bfloat16
FP8 = mybir.dt.float8e4
AF = mybir.ActivationFunctionType
ALU = mybir.AluOpType
DR = mybir.MatmulPerfMode.DoubleRow

N = 1024          # nodes / sequence
C = 768           # model dim
CG = 384          # channels per head-group (6 heads x 64)
D = 64            # head dim
HG = 6            # heads per group
VW = D + 1        # v columns + ones column
VCH = HG * VW     # 390 vplus cols per n-chunk
NT = N // 128     # node-axis 128-chunks
KT = C // 128     # contraction chunks for qkv
BAND = 160        # mask band: |n-m| <= 160
WIN = 448         # 128 + 2*BAND, band window per 128-chunk

SX = 16.0         # fp8 scale for x
SW = 2048.0       # fp8 scale for W_qkv
P1SCALE = 1.0 / (SX * SW)

W0 = [max(0, 128 * mc - BAND) for mc in range(NT)]
W1 = [min(N, 128 * mc + 128 + BAND) for mc in range(NT)]


def build_nc():
    nc = bacc.Bacc("TRN2", target_bir_lowering=False, debug=False)

    xr8_d = nc.dram_tensor("xr8", [2 * C, N], FP8, kind="ExternalInput")
    wr8_d = nc.dram_tensor("wr8", [2 * C, 3 * CG], FP8, kind="ExternalInput")
    gb_d = nc.dram_tensor("gb", [N, WIN], BF16, kind="ExternalInput")
    mtb_d = nc.dram_tensor("mtb", [N, WIN], BF16, kind="ExternalInput")
    w2_d = nc.dram_tensor("wout", [CG, C], BF16, kind="ExternalInput")
    id_d = nc.dram_tensor("ident", [128, 128], BF16, kind="ExternalInput")
    y_d = nc.dram_tensor("y", [N, C], BF16, kind="ExternalOutput")

    with ExitStack() as ctx:
        tc = ctx.enter_context(tile.TileContext(nc))
        persist = ctx.enter_context(tc.tile_pool(name="persist", bufs=1))

        # fp8 x/residual: [p, h(2: x8|r8), kc(6), n(1024)]
        xr8 = persist.tile([128, 2 * KT * N], FP8)
        # fp8 W/residual: [p, h(2: rw8|w8), kc(6), e(1152)]
        wr8 = persist.tile([128, 2 * KT * 3 * CG], FP8)
        q_nm = persist.tile([128, NT * CG], BF16)
        k_nm = persist.tile([128, NT * CG], BF16)
        vplus = persist.tile([128, NT * VCH], BF16)
        qT = persist.tile([128, 3 * N], BF16)
        kT = persist.tile([128, 3 * N], BF16)
        gbs = persist.tile([128, NT * WIN], BF16)
        mtbs = persist.tile([128, NT * WIN], BF16)
        w2 = persist.tile([128, 3 * C], BF16)
        idt = persist.tile([128, 128], BF16)
        at = [persist.tile([128, NT * WIN], BF16, name=f"at{h}")
              for h in range(HG)]

        def at_ap(h, lo, hi):
            return at[h][:, lo:hi]
        zrec = persist.tile([128, NT * HG], F32)

        # DR-pair views of the fp8 operands
        # main term: planes (x8[kc], x8[kc+1]) x (w8[kc], w8[kc+1])
        xm_v = xr8[:].rearrange("p (h t pr n) -> p h t pr n",
                                h=2, t=3, pr=2, n=N)
        wm_v = wr8[:].rearrange("p (h t pr e) -> p h t pr e",
                                h=2, t=3, pr=2, e=3 * CG)
        # correction term: planes (x8[kc], r8[kc]) x (rw8[kc], w8[kc])
        xc_v = xr8[:].rearrange("p (h r) -> p h r", h=2)
        wc_v = wr8[:].rearrange("p (h r) -> p h r", h=2)

        # Input DMAs on sync (HWDGE).  Order is arrival order: the main
        # P1 term only needs x8 + w8; residuals follow; band buffers and
        # W_out land last.
        xr_v = xr8[:].rearrange("p (h kc n) -> p h kc n", h=2, n=N)
        xrd_v = xr8_d[:].rearrange("(h kc p) n -> p h kc n", p=128, h=2)
        wr_v = wr8[:].rearrange("p (h kc e) -> p h kc e", h=2, e=3 * CG)
        wrd_v = wr8_d[:].rearrange("(h kc p) e -> p h kc e", p=128, h=2)

        nc.sync.dma_start(idt[:], id_d[:])
        # x8 kc01, w8 kc01, x8 rest, w8 rest
        nc.sync.dma_start(xr_v[:, 0, 0:2, :], xrd_v[:, 0, 0:2, :])
        nc.sync.dma_start(wr_v[:, 1, 0:2, :], wrd_v[:, 1, 0:2, :])
        nc.sync.dma_start(xr_v[:, 0, 2:KT, :], xrd_v[:, 0, 2:KT, :])
        nc.sync.dma_start(wr_v[:, 1, 2:KT, :], wrd_v[:, 1, 2:KT, :])
        # residuals
        nc.sync.dma_start(xr_v[:, 1, 0:2, :], xrd_v[:, 1, 0:2, :])
        nc.sync.dma_start(wr_v[:, 0, 0:2, :], wrd_v[:, 0, 0:2, :])
        nc.sync.dma_start(xr_v[:, 1, 2:KT, :], xrd_v[:, 1, 2:KT, :])
        nc.sync.dma_start(wr_v[:, 0, 2:KT, :], wrd_v[:, 0, 2:KT, :])
        gb_v = gb_d[:].rearrange("(j p) w -> p j w", p=128)
        nc.sync.dma_start(gbs[:].rearrange("p (j w) -> p j w", j=NT), gb_v)
        mtb_v = mtb_d[:].rearrange("(j p) w -> p j w", p=128)
        nc.sync.dma_start(mtbs[:].rearrange("p (j w) -> p j w", j=NT), mtb_v)
        w2_v = w2_d[:].rearrange("(ds p) e -> p ds e", p=128)
        nc.sync.dma_start(w2[:].rearrange("p (ds e) -> p ds e", ds=3), w2_v)
        for j in range(NT):
            vch = vplus[:, j * VCH:(j + 1) * VCH].rearrange(
                "p (h w) -> p h w", w=VW)
            nc.gpsimd.memset(vch[:, :, D:VW], 1.0)

        # ================= Phase 1: qkv projection (fp8 DR) ==========
        def p1_block(p, waves=range(4), after_wave=None):
            for w in waves:
                accs = {}
                for jj in range(2):
                    accs[jj] = ps1.tile([128, CG], F32, tag=f"qkv{jj}",
                                        name=f"acc{jj}")
                # main term: 3 DR matmuls pairing kc chunks
                for t in range(3):
                    for jj in range(2):
                        j = w * 2 + jj
                        nc.tensor.matmul(
                            accs[jj][:],
                            xm_v[:, 0, t, :, j * 128:(j + 1) * 128],
                            wm_v[:, 1, t, :, p * CG:(p + 1) * CG],
                            start=(t == 0), stop=False, perf_mode=DR)
                # corrections: one DR per kc (planes x8.rw8 + r8.w8)
                for kc in range(KT):
                    for jj in range(2):
                        j = w * 2 + jj
                        nc.tensor.matmul(
                            accs[jj][:],
                            xc_v[:, :, kc * N + j * 128:
                                 kc * N + (j + 1) * 128],
                            wc_v[:, :, kc * 3 * CG + p * CG:
                                 kc * 3 * CG + (p + 1) * CG],
                            start=False, stop=(kc == KT - 1), perf_mode=DR)
                for jj in range(2):
                    j = w * 2 + jj
                    acc = accs[jj]
                    on_dve = (jj == 0)
                    if p < 2:
                        dst = (q_nm if p == 0 else k_nm)[:, j * CG:
                                                         (j + 1) * CG]
                        if on_dve:
                            nc.vector.tensor_scalar(
                                dst, acc[:], P1SCALE, 0.0,
                                op0=ALU.mult, op1=ALU.max)
                        else:
                            nc.scalar.activation(dst, acc[:], AF.Relu,
                                                 scale=P1SCALE)
                    else:
                        vch = vplus[:, j * VCH:(j + 1) * VCH].rearrange(
                            "p (h w) -> p h w", w=VW)
                        if on_dve:
                            nc.vector.tensor_scalar_mul(
                                vch[:, :, 0:D],
                                acc[:].rearrange("p (h w) -> p h w", w=D),
                                P1SCALE)
                        else:
                            nc.scalar.activation(
                                vch[:, :, 0:D],
                                acc[:].rearrange("p (h w) -> p h w", w=D),
                                AF.Copy, scale=P1SCALE)
                if after_wave is not None:
                    after_wave(w)

        # ============ Phases 2-4 (banded, chunk-interleaved) ============
        with tc.tile_pool(name="st_ps", bufs=3, space="PSUM") as st_pool, \
             tc.tile_pool(name="stsb", bufs=3) as stsb_pool:

            def emit_st_at(h, mc):
                """S^T over the band window of m-chunk mc, then mask it."""
                ds, r0 = h // 2, (h % 2) * D
                w0, w1 = W0[mc], W1[mc]
                wd = w1 - w0
                st = st_pool.tile([128, 512], F32, tag="st")
                nc.tensor.matmul(
                    st[:, 0:wd],
                    kT[r0:r0 + D, ds * N + mc * 128: ds * N + (mc + 1) * 128],
                    qT[r0:r0 + D, ds * N + w0: ds * N + w1],
                    start=True, stop=True)
                if h >= 4 and mc <= 4:
                    stsb = stsb_pool.tile([128, WIN], BF16, tag="stsb")
                    nc.scalar.activation(stsb[:, 0:wd], st[:, 0:wd], AF.Copy)
                    eng = nc.vector if h == 4 else nc.gpsimd
                    eng.tensor_tensor(
                        at_ap(h, mc * WIN, mc * WIN + wd),
                        stsb[:, 0:wd],
                        mtbs[:, mc * WIN: mc * WIN + wd], op=ALU.mult)
                else:
                    nc.vector.tensor_tensor(
                        at_ap(h, mc * WIN, mc * WIN + wd),
                        st[:, 0:wd],
                        mtbs[:, mc * WIN: mc * WIN + wd], op=ALU.mult)

            # ---- Phase 1 + 2 interleaved (P1 pool closes before P3) ----
            with tc.tile_pool(name="ps1", bufs=1, space="PSUM") as ps1, \
                 tc.tile_pool(name="gps", bufs=3, space="PSUM") as gps:

                def p2_quarter(qq, src, dstT):
                    m0 = 256 * qq
                    order = [2 * qq] + [
                        x for x in (2 * qq - 2, 2 * qq - 1,
                                    2 * qq + 2, 2 * qq + 3)
                        if 0 <= x < NT] + [2 * qq + 1]
                    spans = [(jj, max(m0, W0[jj]), min(m0 + 256, W1[jj]))
                             for jj in order]
                    spans = [s for s in spans if s[1] < s[2]]
                    for ds in range(3):
                        g_acc = gps.tile([128, 256], F32, tag="g")
                        for ii, (jj, a, b) in enumerate(spans):
                            nc.tensor.matmul(
                                g_acc[:, a - m0: b - m0],
                                src[:, jj * CG + ds * 128:
                                    jj * CG + (ds + 1) * 128],
                                gbs[:, jj * WIN + a - W0[jj]:
                                    jj * WIN + b - W0[jj]],
                                start=(ii == 0),
                                stop=(ii == len(spans) - 1))
                        dst = dstT[:, ds * N + m0: ds * N + m0 + 256]
                        nc.scalar.activation(dst, g_acc[:], AF.Copy)

                def q_after_wave(w):
                    if w == 1:
                        p2_quarter(0, q_nm, qT)
                    elif w == 2:
                        p2_quarter(1, q_nm, qT)
                    elif w == 3:
                        p2_quarter(2, q_nm, qT)
                        p2_quarter(3, q_nm, qT)

                def k_after_wave(w):
                    if w == 1:
                        p2_quarter(0, k_nm, kT)
                    elif w == 2:
                        p2_quarter(1, k_nm, kT)
                        for h in range(HG):
                            for mc in range(2):
                                emit_st_at(h, mc)

                p1_block(0, after_wave=q_after_wave)  # q projection
                p1_block(1, after_wave=k_after_wave)  # k projection

                p2_quarter(2, k_nm, kT)
                for h in range(HG):
                    emit_st_at(h, 2)
                p2_quarter(3, k_nm, kT)
                p1_block(2)                     # v projection

            # ---- Phases 3+4, P4 one chunk behind P3 ----
            with tc.tile_pool(name="o_ps", bufs=2, space="PSUM") as o_pool, \
                 tc.tile_pool(name="tp_ps", bufs=1, space="PSUM") as tp_pool, \
                 tc.tile_pool(name="y_ps", bufs=1, space="PSUM") as y_pool, \
                 tc.tile_pool(name="ot_sb", bufs=2) as ot_pool, \
                 tc.tile_pool(name="osb_sb", bufs=6) as osb_pool, \
                 tc.tile_pool(name="ysb_sb", bufs=2) as ysb_pool:

                osb_ring = []
                ot_ring = []

                def emit_tr(j):
                    """Transpose chunk j's scaled O to d-major."""
                    osb3 = osb_ring.pop(0)
                    tp = tp_pool.tile([128, 3 * 128], BF16, tag="tp")
                    for g2 in range(3):
                        nc.tensor.transpose(
                            tp[:, g2 * 128:(g2 + 1) * 128], osb3[g2][:],
                            idt[:])
                    otTj = ot_pool.tile([128, 3 * 128], BF16, tag="otj")
                    nc.scalar.activation(otTj[:], tp[:], AF.Copy)
                    ot_ring.append(otTj)

                def emit_p4(j, last=False):
                    """Output projection for chunk j."""
                    otTj = ot_ring.pop(0)
                    yp = y_pool.tile([128, C], F32, tag="y")
                    for g2 in range(3):
                        for e0, ew in ((0, 512), (512, 256)):
                            nc.tensor.matmul(
                                yp[:, e0:e0 + ew],
                                otTj[:, g2 * 128: (g2 + 1) * 128],
                                w2[:, g2 * C + e0: g2 * C + e0 + ew],
                                start=(g2 == 0), stop=(g2 == 2))
                    ysb = ysb_pool.tile([128, C], BF16, tag="ysb")
                    if last:
                        nc.scalar.activation(ysb[:, 0:384], yp[:, 0:384],
                                             AF.Copy)
                        nc.sync.dma_start(y_d[j * 128:(j + 1) * 128, 0:384],
                                          ysb[:, 0:384])
                        nc.vector.tensor_copy(ysb[:, 384:C], yp[:, 384:C])
                        nc.sync.dma_start(y_d[j * 128:(j + 1) * 128, 384:C],
                                          ysb[:, 384:C])
                    else:
                        nc.scalar.activation(ysb[:], yp[:], AF.Copy)
                        nc.sync.dma_start(y_d[j * 128:(j + 1) * 128, :],
                                          ysb[:])

                for j in range(NT):
                    o_ps = o_pool.tile([128, 512], F32, tag="o")
                    tail = j + 1 if j + 1 < NT else j - 1
                    mcs = [j] + [m for m in (j - 1, j + 1, j - 2, j + 2)
                                 if 0 <= m < NT and m != tail] + [tail]
                    for h in range(HG):
                        for ii, mc in enumerate(mcs):
                            a = max(128 * j, W0[mc])
                            b = min(128 * (j + 1), W1[mc])
                            nc.tensor.matmul(
                                o_ps[a - 128 * j: b - 128 * j,
                                     h * VW: (h + 1) * VW],
                                at_ap(h, mc * WIN + a - W0[mc],
                                      mc * WIN + b - W0[mc]),
                                vplus[:, mc * VCH + h * VW:
                                      mc * VCH + (h + 1) * VW],
                                start=(h == 0 and ii == 0),
                                stop=(h == HG - 1 and ii == len(mcs) - 1),
                                tile_position=(0, a - 128 * j),
                                skip_group_check=True)
                    zr = zrec[:, j * HG: (j + 1) * HG]
                    nc.vector.reciprocal(
                        zr.rearrange("p (h o) -> p h o", o=1),
                        o_ps[:, 0:VCH].rearrange(
                            "p (h w) -> p h w", w=VW)[:, :, D:VW])
                    osb3 = []
                    for g2 in range(3):
                        osb = osb_pool.tile([128, 128], BF16, tag="osb")
                        for hh in range(2):
                            h = 2 * g2 + hh
                            if j >= 5:
                                nc.vector.tensor_scalar_mul(
                                    osb[:, hh * D: (hh + 1) * D],
                                    o_ps[:, h * VW: h * VW + D],
                                    zr[:, h: h + 1])
                            else:
                                nc.scalar.activation(
                                    osb[:, hh * D: (hh + 1) * D],
                                    o_ps[:, h * VW: h * VW + D],
                                    AF.Copy, scale=zr[:, h: h + 1])
                        osb3.append(osb)
                    osb_ring.append(osb3)
                    if j + 3 < NT:
                        for h in range(HG):
                            emit_st_at(h, j + 3)
                    if j > 0:
                        emit_tr(j - 1)
                    if j > 1:
                        emit_p4(j - 2)
                emit_tr(NT - 1)
                emit_p4(NT - 2)
                emit_p4(NT - 1)

    nc.compile()
    return nc


_NC_CACHE = {}


def _get_nc():
    if "nc" not in _NC_CACHE:
        _NC_CACHE["nc"] = build_nc()
    return _NC_CACHE["nc"]


def make_in_maps(x, W_qkv, W_out, mask):
    import ml_dtypes
    bf = ml_dtypes.bfloat16
    e4 = ml_dtypes.float8_e4m3
    mask = np.asarray(mask, dtype=np.float32)
    G = np.eye(N, dtype=np.float32) + 0.1 * mask
    maskT = np.ascontiguousarray(mask.T)
    gbp = np.zeros((N, WIN), dtype=np.float32)
    mtbp = np.zeros((N, WIN), dtype=np.float32)
    for j in range(NT):
        w0, w1 = W0[j], W1[j]
        gbp[j * 128:(j + 1) * 128, 0:w1 - w0] = G[j * 128:(j + 1) * 128, w0:w1]
        mtbp[j * 128:(j + 1) * 128, 0:w1 - w0] = \
            maskT[j * 128:(j + 1) * 128, w0:w1]
    gbp = gbp.astype(bf)
    mtbp = mtbp.astype(bf)
    ident = np.eye(128, dtype=np.float32).astype(bf)
    in_maps = []
    for c in range(8):
        b, g = divmod(c, 2)
        xTb = np.ascontiguousarray(x[b].T) * SX
        x8 = xTb.astype(e4)
        r8 = (xTb - x8.astype(np.float32)).astype(e4)
        xr8 = np.concatenate([x8, r8], axis=0)
        wq = W_qkv[:, g * CG:(g + 1) * CG]
        wk = W_qkv[:, C + g * CG: C + (g + 1) * CG]
        wv = W_qkv[:, 2 * C + g * CG: 2 * C + (g + 1) * CG]
        w = np.ascontiguousarray(np.concatenate([wq, wk, wv], axis=1)) * SW
        w8 = w.astype(e4)
        rw8 = (w - w8.astype(np.float32)).astype(e4)
        # wr8 layout: [rw8 ; w8] so DR corr planes are (x8*rw8, r8*w8)
        wr8 = np.concatenate([rw8, w8], axis=0)
        w2p = np.ascontiguousarray(W_out[g * CG:(g + 1) * CG, :])
        in_maps.append({"xr8": xr8, "wr8": wr8,
                        "gb": gbp, "mtb": mtbp,
                        "wout": w2p.astype(bf), "ident": ident})
    return in_maps


def kernel(x, W_qkv, W_out, b_out, mask, _trace=False):
    x = np.asarray(x, dtype=np.float32)
    W_qkv = np.asarray(W_qkv, dtype=np.float32)
    W_out = np.asarray(W_out, dtype=np.float32)
    b_out = np.asarray(b_out, dtype=np.float32)
    mask = np.asarray(mask, dtype=np.float32)

    nc = _get_nc()
    in_maps = make_in_maps(x, W_qkv, W_out, mask)
    res = run_bass_kernel_spmd(nc, in_maps, core_ids=list(range(8)),
                               trace=_trace)
    parts = [r["y"] for r in res.results]
    out = np.empty((4, N, C), dtype=np.float32)
    for b in range(4):
        out[b] = (parts[2 * b].astype(np.float32)
                  + parts[2 * b + 1].astype(np.float32) + b_out)
    if _trace:
        kernel._last_results = res
    return out
